# revision 11
# baseline (speedup 1.0000x reference)
"""LocallyHierarchicalNet Trainium2 kernel.

Net: 10 locally-connected conv1d layers (kernel=stride=2, unshared weights
per position), B=128, C_in=3, H=256, D=1024, then mean + linear head.

Strategy (8 NeuronCores, SPMD):
  - Position-shard layers 0-6: core i owns output positions [i*64,(i+1)*64)
    of layer 0, which narrows to exactly 1 position at layer 6 with zero
    cross-core traffic (binary-tree locality).
  - AllGather the 8 layer-6 outputs (256x128 f32 each) on-chip, then every
    core redundantly computes layers 7-9 + head (tiny).
  - Matmul layout: activations live as [C on partitions, B on free] per
    position; per output position out[B,O] = sum_k X_chunk.T @ W_chunk with
    X chunks stationary (lhsT) and host-pretransposed weights moving
    (rhs [K,O=256]) in float32r (FP22, 1 cyc/row at N>=256).  Output [B,256]
    gets ReLU'd on ScalarE, then 2 PE transposes restore [C,B] chain layout.
  - Weights stream from HBM in multi-MB contiguous slabs, double buffered.
"""

import sys

sys.path.insert(0, "/opt/trn_rl_repo")

import numpy as np

N_CORES = 8
B = 128
C_IN = 3
H = 256
OUT = 10

# per-core output positions per layer (layers 1..9 use 128-partition slabs)
NPOS = {1: 32, 2: 16, 3: 8, 4: 4, 5: 2, 6: 1, 7: 4, 8: 2, 9: 1}
# weight slab size (positions per DMA) per layer
SLAB = {1: 4, 2: 4, 3: 4, 4: 4, 5: 2, 6: 1, 7: 4, 8: 2, 9: 1}

_NC = None


def _build():
    import concourse.bacc as bacc
    import concourse.mybir as mybir
    import concourse.tile as tile
    from concourse.masks import make_identity

    dt = mybir.dt
    f32 = dt.float32
    f32r = dt.float32r
    Relu = mybir.ActivationFunctionType.Relu
    Copy = mybir.ActivationFunctionType.Copy

    nc = bacc.Bacc(
        "TRN2", target_bir_lowering=False, debug=False, num_devices=N_CORES
    )
    dma_engines = [nc.sync, nc.scalar, nc.gpsimd]
    dma_rr = [0]

    def dma_q(out, in_):
        eng = dma_engines[dma_rr[0] % len(dma_engines)]
        dma_rr[0] += 1
        eng.dma_start(out, in_)

    x0_d = nc.dram_tensor("x0", [6, 64, B], f32, kind="ExternalInput")
    w_d = {0: nc.dram_tensor("w0", [6, 64, H], f32, kind="ExternalInput")}
    for l in range(1, 10):
        w_d[l] = nc.dram_tensor(
            f"w{l}", [128, NPOS[l] * 1024], f32, kind="ExternalInput"
        )
    beta_d = nc.dram_tensor("beta", [H, OUT], f32, kind="ExternalInput")
    out_d = nc.dram_tensor("out", [B, OUT], f32, kind="ExternalOutput")

    with tile.TileContext(nc) as tc:
        with (
            tc.tile_pool(name="sb", bufs=1) as sb,
            tc.tile_pool(name="wp", bufs=2) as wp,
            tc.tile_pool(name="yp", bufs=4) as yp,
            tc.tile_pool(name="psp", bufs=3, space="PSUM") as psp,
            tc.tile_pool(name="ptp", bufs=4, space="PSUM") as ptp,
            tc.tile_pool(name="dram", bufs=1, space="DRAM") as dp,
        ):
            ident = sb.tile([128, 128], f32, tag="ident", name="ident")
            make_identity(nc, ident)

            beta_sb = sb.tile([128, 2 * OUT], f32r, tag="beta", name="beta_sb")
            nc.sync.dma_start(beta_sb[:, 0:OUT], beta_d[0:128, :].bitcast(f32r))
            nc.sync.dma_start(beta_sb[:, OUT : 2 * OUT], beta_d[128:256, :].bitcast(f32r))

            x0 = sb.tile([6, 64, B], f32r, tag="x0", name="x0_sb")
            nc.sync.dma_start(x0[:], x0_d[:].bitcast(f32r))

            # ---- layer 0: C_in=3, K=6; lhsT = w0 [6, O-chunk], rhs = x [6, B]
            # output written directly in chain layout [O, B] (no transpose).
            X1 = sb.tile([128, 64, 2, B], f32r, tag="xo", name="X1")
            s3 = 1.0 / (3.0**0.5)
            for s in range(8):  # 8 slabs x 8 positions
                w0s = wp.tile([6, 8, H], f32r, tag="w0s", name=f"w0s_{s}")
                dma_q(w0s[:], w_d[0][:, s * 8 : (s + 1) * 8, :].bitcast(f32r))
                for pp in range(8):
                    pos = s * 8 + pp
                    pt = ptp.tile([128, 2, B], f32, tag="pt", name=f"p0_{pos}")
                    for j in range(2):
                        nc.tensor.matmul(
                            pt[:, j, :],
                            w0s[:, pp, j * 128 : (j + 1) * 128],
                            x0[:, pos, :],
                            start=True,
                            stop=True,
                        )
                    if pos % 2 == 0:
                        nc.scalar.activation(
                            X1[:, pos, :, :], pt[:], Relu, scale=s3
                        )
                    else:
                        nc.vector.tensor_scalar(
                            X1[:, pos, :, :], pt[:], s3, 0.0,
                            mybir.AluOpType.mult, mybir.AluOpType.max,
                        )

            def lc_layer(l, Xin, xtag):
                """One locally-connected layer l>=1 (C=256, K=512, O=256)."""
                n = NPOS[l]
                Xout = sb.tile([128, n, 2, B], f32r, tag=xtag, name=f"X{l + 1}")
                slab = SLAB[l]

                def flush(p, yb):
                    pt = ptp.tile([128, 2, B], f32, tag="pt", name=f"pt{l}_{p}")
                    for j in range(2):
                        nc.tensor.transpose(
                            pt[:, j, :],
                            yb[:, j * 128 : (j + 1) * 128],
                            ident[:],
                        )
                    nc.scalar.copy(Xout[:, p, :, :], pt[:])

                pend = None
                for s in range(n // slab):
                    ws = wp.tile([128, slab * 1024], f32r, tag="ws", name=f"ws{l}_{s}")
                    dma_q(
                        ws[:], w_d[l][:, s * slab * 1024 : (s + 1) * slab * 1024].bitcast(f32r)
                    )
                    for pp in range(slab):
                        p = s * slab + pp
                        ps = psp.tile([128, H], f32, tag="ps", name=f"ps{l}_{p}")
                        for kk in range(2):
                            for ch in range(2):
                                ci = kk * 2 + ch
                                c0 = (pp * 4 + ci) * 256
                                nc.tensor.matmul(
                                    ps[:],
                                    Xin[:, 2 * p + kk, ch, :],
                                    ws[:, c0 : c0 + 256],
                                    start=(ci == 0),
                                    stop=(ci == 3),
                                )
                        yb = yp.tile([128, H], f32, tag="yb", name=f"yb{l}_{p}")
                        nc.vector.tensor_scalar(
                            yb[:], ps[:], 1.0 / 16.0, 0.0,
                            mybir.AluOpType.mult, mybir.AluOpType.max,
                        )
                        if pend is not None:
                            flush(*pend)
                        pend = (p, yb)
                flush(*pend)
                return Xout

            X = X1
            for l, xtag in [(1, "xe"), (2, "xo2"), (3, "xe"), (4, "xo2"), (5, "xe"), (6, "xown")]:
                X = lc_layer(l, X, xtag)

            # ---- AllGather the single layer-6 output position across cores
            ag_in = dp.tile([H, B], f32, name="ag_in")
            ag_out = dp.tile(
                [N_CORES * H, B], f32, addr_space="Shared", name="ag_out"
            )
            nc.sync.dma_start(
                ag_in.rearrange("(ch p) b -> p ch b", ch=2),
                X[:, 0, :, :].bitcast(f32),
            )
            nc.gpsimd.collective_compute(
                "AllGather",
                mybir.AluOpType.bypass,
                replica_groups=[list(range(N_CORES))],
                ins=[ag_in.opt()],
                outs=[ag_out.opt()],
            )
            X7 = sb.tile([128, 8, 2, B], f32r, tag="x7", name="X7")
            nc.sync.dma_start(
                X7[:],
                ag_out.rearrange("(pos ch p) b -> p pos ch b", pos=8, ch=2).bitcast(f32r),
            )

            X = X7
            for l, xtag in [(7, "xo2"), (8, "xe"), (9, "xo2")]:
                X = lc_layer(l, X, xtag)

            # ---- head: out[b, j] = sum_c X10[c, b] * beta[c, j] / 256
            ph = psp.tile([128, OUT], f32, tag="ps", name="ph")
            for ch in range(2):
                nc.tensor.matmul(
                    ph[:],
                    X[:, 0, ch, :],
                    beta_sb[:, ch * OUT : (ch + 1) * OUT],
                    start=(ch == 0),
                    stop=(ch == 1),
                )
            ob = yp.tile([128, OUT], f32, tag="ob", name="ob")
            nc.scalar.activation(ob[:], ph[:], Copy, scale=1.0 / 256.0)
            nc.sync.dma_start(out_d[:], ob[:])

    nc.compile()
    return nc


def _get_nc():
    global _NC
    if _NC is None:
        _NC = _build()
    return _NC


def _prep(inputs):
    x = np.asarray(inputs["x"], dtype=np.float32)
    beta = np.asarray(inputs["beta"], dtype=np.float32)
    ws = [np.asarray(inputs[f"w{l}"], dtype=np.float32) for l in range(10)]

    # x (B,3,1024) -> (kk=2, c=3, d=512, b)
    xk = np.ascontiguousarray(x.reshape(B, 3, 512, 2).transpose(3, 1, 2, 0))
    # w0 (256,3,512,2) -> (kk, c, d, o)
    w0t = np.ascontiguousarray(ws[0].transpose(3, 1, 2, 0))

    # wl (256,256,dl,2) -> slab (pp=128, (pos, kk, ch, o))
    slabs = {}
    for l in range(1, 10):
        w = ws[l]
        dl = w.shape[2]
        wt = w.transpose(1, 2, 3, 0)  # (c, dl, kk, o)
        wt = wt.reshape(2, 128, dl, 2, 256).transpose(1, 2, 3, 0, 4)
        slabs[l] = np.ascontiguousarray(wt).reshape(128, dl * 1024)

    in_maps = []
    for i in range(N_CORES):
        m = {
            "x0": np.ascontiguousarray(xk[:, :, i * 64 : (i + 1) * 64, :]).reshape(
                6, 64, B
            ),
            "w0": np.ascontiguousarray(w0t[:, :, i * 64 : (i + 1) * 64, :]).reshape(
                6, 64, H
            ),
            "beta": beta,
        }
        for l in range(1, 7):
            n = NPOS[l]
            m[f"w{l}"] = np.ascontiguousarray(
                slabs[l][:, i * n * 1024 : (i + 1) * n * 1024]
            )
        for l in range(7, 10):
            m[f"w{l}"] = slabs[l]
        in_maps.append(m)
    return in_maps


def _run(in_maps, trace=False):
    from concourse import bass_utils

    return bass_utils.run_bass_kernel_spmd(
        _get_nc(), in_maps, core_ids=list(range(N_CORES)), trace=trace
    )


def kernel(**inputs):
    res = _run(_prep(inputs))
    return np.asarray(res.results[0]["out"], dtype=np.float32)
